# revision 14
# baseline (speedup 1.0000x reference)
"""MoE QLoRA linear kernel for Trainium2 (8 NeuronCores).

Computes, for x:(B,S,IN) f32:
    base  = x @ W.T + b
    gates = softmax(x @ Wr.T)                       # (tok, E)
    proj  = x @ A[e].T                              # (tok, E, R)
    out   = base + sum_e SCALE * gates[...,e] * (proj[...,e,:] @ Bm[e].T)

Sharding: 2D — core c owns token group g=c//2 (2048 tokens) and out-feature
half h=c%2 (2048 features).  The 2048-token slice gives each W stationary
tile FOUR consecutive 512-token matmuls instead of two.  Measured on this
hardware, a stream of same-stationary matmuls chained behind one LDWEIGHTS
costs 53ns (weight load / pipeline drain, they hide each other) + 213ns per
matmul, so chain length S amortizes the 53ns: S=2 -> 266ns/MM (the v1
kernel), S=4 -> 226ns/MM.  The cost of the 2D sharding is that phase 1
(router+proj, ~6% of the flops) is computed twice (once per feature-half).

Algebraic folds (as in v1): the gated expert mix is a single rank-(E*R+1)
matmul with the bias folded in as a ones-row contraction; all matmul inputs
fp16 (fp32 PSUM accumulation).

Per-core kernel:
  phase 1: PSUM(36,512)x4 = [A;Wr]^T-stationary matmuls, k-outer/slab-inner
           (4-chase per k); softmax via ACT exp + tiny PE reduction matmuls;
           wproj written fp16.  o-tile-0's k-loop for slabs 0-2 is emitted
           between the proj matmuls and the gating chain so the PE stays
           busy while ACT/DVE run softmax (slab 3's PSUM tag is
           single-buffered to fit 8 banks, so its o-tile-0 k-loop runs
           after gating frees pp3).
  phase 2: per o-tile (16): W-tile-stationary k-loop, 4-chase over slabs,
           + lora matmul (k=33) accumulated in PSUM; slab-3-first copies so
           the single-buffered pp3 tag never stalls the next o-tile.

Post-compile passes (both validated on hardware):
  prune_mm_updates  — drop per-matmul sem updates nobody waits on (a sem
                      write stalls a bare MM->MM turnaround ~40ns).
  dedup_ldweights   — drop reloads of the already-loaded stationary; this
                      is what lets same-stationary matmuls chain (an
                      LDWEIGHTS between them forces a full array drain).
"""

import numpy as np

import concourse.bass as bass
import concourse.tile as tile
from concourse import bacc, mybir
from concourse import bass_utils

# Problem shape (hardcoded; kernel.py must be self-contained)
B, S, IN, OUT, E, R = 4, 2048, 4096, 4096, 4, 8
SCALE = 16.0 / 8.0
N_CORES = 8
TOK = B * S                  # 8192 tokens
TG = 4                       # token groups
FH = 2                       # feature halves
TPC = TOK // TG              # 2048 tokens per core
OPC = OUT // FH              # 2048 out features per core
P = 128                      # partitions
KT = IN // P                 # 32 k-tiles (contraction)
OT = OPC // P                # 16 output tiles per core
NSLAB = 512                  # moving-operand free size (PSUM bank = 512 f32)
NS = TPC // NSLAB            # 4 token slabs per core
ER = E * R                   # 32 low-rank rows
ERA = ER + 1                 # +1 ones row (bias fold)

F16 = mybir.dt.float16
F32 = mybir.dt.float32

_NC = None

# Optional post-compile transform hook (used by experiments; None in prod).
POST_COMPILE = None


def build_nc(reps=1):
    nc = bacc.Bacc("TRN2", target_bir_lowering=False, debug=False)

    xd = nc.dram_tensor("xd", [P, KT, TPC], F16, kind="ExternalInput")
    wd = nc.dram_tensor("wd", [OT, P, KT, P], F16, kind="ExternalInput")
    artd = nc.dram_tensor("artd", [P, KT, ER + E], F16, kind="ExternalInput")
    btd = nc.dram_tensor("btd", [ERA, OPC], F16, kind="ExternalInput")
    seld = nc.dram_tensor("seld", [E, ER], F32, kind="ExternalInput")
    od = nc.dram_tensor("od", [OPC, TPC], F32, kind="ExternalOutput")

    with tile.TileContext(nc) as tc:
        with (
            tc.tile_pool(name="consts", bufs=1) as consts,
            tc.tile_pool(name="wpool", bufs=3) as wpool,
            tc.tile_pool(name="opool", bufs=3) as opool,
            tc.tile_pool(name="small", bufs=2) as small,
            tc.tile_pool(name="psum", bufs=1, space="PSUM") as psum,
        ):
            art_sb = consts.tile([P, KT, ER + E], F16)
            nc.sync.dma_start(out=art_sb[:], in_=artd[:])
            bt_sb = consts.tile([ERA, OPC], F16)
            nc.sync.dma_start(out=bt_sb[:], in_=btd[:])
            sel_sb = consts.tile([E, ER], F32)
            nc.sync.dma_start(out=sel_sb[:], in_=seld[:])

            w_tiles = {}

            def load_w(ot):
                w_sb = wpool.tile([P, KT, P], F16, tag="w", name="w_sb")
                nc.sync.dma_start(out=w_sb[:], in_=wd[ot])
                w_tiles[ot] = w_sb

            # first two W tiles before the bulk x load: o-tile 0 can start
            # as soon as phase-1 finishes on the PE
            load_w(0)
            load_w(1)

            # Resident activations: x^T tiled (p=i%128, k=i//128, t), fp16.
            x_sb = consts.tile([P, KT, TPC], F16)
            for k in range(KT):
                nc.sync.dma_start(out=x_sb[:, k, :], in_=xd[:, k, :])

            ones_e1 = consts.tile([E, 1], F32)
            nc.vector.memset(ones_e1[:], 1.0)
            ones_1e = consts.tile([1, E], F32)
            nc.vector.memset(ones_1e[:], 1.0)
            # Gated low-rank projection, fp16, rows 0..31 = wproj, row 32 = ones.
            wp_sb = consts.tile([ERA, TPC], F16)
            nc.vector.memset(wp_sb[ER : ER + 1, :], 1.0)

            def tsl(t):
                return slice(t * NSLAB, (t + 1) * NSLAB)

            # ---------- phase 1: proj + router matmuls ----------
            # k-outer / slab-inner: each art k-tile is loaded once and
            # chased by NS matmuls.
            def bank_tile(shape, t, name):
                # tags b0..b2 double-buffered, b3 single, gat single: 8 banks
                return psum.tile(
                    shape, F32, tag=f"b{t}",
                    bufs=(2 if t < NS - 1 else 1), name=name,
                )

            pps = [
                bank_tile([ER + E, NSLAB], t, f"pp{t}") for t in range(NS)
            ]
            for k in range(KT):
                for t in range(NS):
                    nc.tensor.matmul(
                        pps[t][:],
                        art_sb[:, k, :],
                        x_sb[:, k, tsl(t)],
                        start=(k == 0),
                        stop=(k == KT - 1),
                    )

            def gating(t):
                # softmax over the 4 expert rows (no max-sub: |logit| < ~8),
                # partition reductions/broadcasts via tiny PE matmuls whose
                # PSUM temporaries live in the free buffer of tag b{t+1}/..
                pp = pps[t]
                e_sb = small.tile([E, NSLAB], F32, tag="e", name="e_sb")
                nc.scalar.activation(
                    e_sb[:], pp[ER : ER + E, :], mybir.ActivationFunctionType.Exp
                )
                s_ps = psum.tile([1, NSLAB], F32, tag="gat", bufs=1, name="s_ps")
                nc.tensor.matmul(s_ps[:], ones_e1[:], e_sb[:])  # sum_e exp
                r_sb = small.tile([1, NSLAB], F32, tag="r", name="r_sb")
                nc.vector.reciprocal(r_sb[:], s_ps[:])
                r4_ps = psum.tile([E, NSLAB], F32, tag="gat", bufs=1, name="r4_ps")
                nc.tensor.matmul(r4_ps[:], ones_1e[:], r_sb[:])  # bcast 4 rows
                g4_sb = small.tile([E, NSLAB], F32, tag="g4", name="g4_sb")
                nc.vector.tensor_mul(g4_sb[:], e_sb[:], r4_ps[:])
                # (SCALE * gate)[er, t] via 0/1*SCALE selection matmul
                g32_ps = psum.tile([ER, NSLAB], F32, tag="gat", bufs=1, name="g32_ps")
                nc.tensor.matmul(g32_ps[:], sel_sb[:], g4_sb[:])
                # walrus: tensor_tensor may read at most one operand from PSUM
                g32_sb = small.tile([ER, NSLAB], F32, tag="g32s", name="g32_sb")
                nc.vector.tensor_copy(g32_sb[:], g32_ps[:])
                nc.vector.tensor_mul(wp_sb[0:ER, tsl(t)], pp[0:ER, :], g32_sb[:])

            # ---------- phase 2 helpers ----------
            def base_kloop(ot, slabs):
                if ot not in w_tiles:
                    load_w(ot)
                pots = {
                    t: bank_tile([P, NSLAB], t, f"po{t}") for t in slabs
                }
                for k in range(KT):
                    for t in slabs:
                        nc.tensor.matmul(
                            pots[t][:],
                            w_tiles[ot][:, k, :],
                            x_sb[:, k, tsl(t)],
                            start=(k == 0),
                            stop=False,
                        )
                return pots

            def base_tail(ot, pots):
                osl = slice(ot * P, (ot + 1) * P)
                # slab NS-1 first: its PSUM tag is single-buffered, so its
                # copy must land before the next o-tile's k-loop reaches it
                order = sorted(pots.keys(), reverse=True)
                for t in order:
                    nc.tensor.matmul(
                        pots[t][:],
                        bt_sb[:, osl],
                        wp_sb[:, tsl(t)],
                        start=False,
                        stop=True,
                    )
                o_sb = opool.tile([P, TPC], F32, tag="o", name="o_sb")
                for t in order:
                    nc.vector.tensor_copy(o_sb[:, tsl(t)], pots[t][:])
                nc.sync.dma_start(out=od[osl, :], in_=o_sb[:])
                del w_tiles[ot]

            # ---------- phase 2 ----------
            for rep in range(reps):
                if rep == 0:
                    # o-tile 0, slabs 0..NS-2 keep the PE busy during the
                    # gating chain; slab NS-1 shares PSUM with pp3 (tag b3
                    # is single-buffered) so its k-loop runs after gating.
                    pots0 = base_kloop(0, range(NS - 1))
                    for t in range(NS):
                        gating(t)
                    pots0.update(base_kloop(0, [NS - 1]))
                    base_tail(0, pots0)
                    start_ot = 1
                else:
                    start_ot = 0
                for ot in range(start_ot, OT):
                    pots = base_kloop(ot, range(NS))
                    base_tail(ot, pots)

    nc.compile()
    if POST_COMPILE is not None:
        POST_COMPILE(nc)
    else:
        prune_mm_updates(nc)
        dedup_ldweights(nc)
    return nc


def dedup_ldweights(nc):
    """Delete redundant consecutive InstLdweights from the PE stream.

    After bacc lowering every matmul is a standalone InstLdweights followed
    by a non-self-loading InstMatmult.  When consecutive matmuls use the
    same stationary tile, the later loads are no-op reloads of weights
    already in the array — and worse, they force a full pipeline drain
    between the matmuls, preventing the fill-chasing that makes
    same-stationary chains cheap.  Deleting one is safe when (a) its full AP
    signature matches the immediately preceding InstLdweights, (b) only
    non-self-loading InstMatmults sit between them, and (c) it carries no
    semaphore waits/updates.  Any rewrite of the underlying SBUF region is
    ordered after the *later* consumers by the tile framework's region
    tracking, so the weight contents cannot change between the two loads.
    """
    from concourse import mybir

    def sig(i):
        ap = i.ins[0]
        return (
            ap.memref,
            ap.offset,
            str(ap.ap),
            str(ap.dtype),
            str(i.tile_position),
            str(i.tile_size),
            str(i.perf_mode),
            str(i.is_transpose),
        )

    removed = 0
    for b in nc.m.functions[0].blocks:
        last = None
        keep = []
        for i in b.instructions:
            if i.engine != mybir.EngineType.PE:
                keep.append(i)
                continue
            if isinstance(i, mybir.InstLdweights):
                si = i.sync_info
                clean = si is None or (not si.on_wait and not si.on_update)
                if clean and last is not None and sig(i) == last:
                    removed += 1
                    continue  # drop the redundant reload
                last = sig(i)
                keep.append(i)
            elif (
                isinstance(i, mybir.InstMatmult)
                and getattr(i, "ldweights", None) is False
            ):
                keep.append(i)
            else:
                last = None
                keep.append(i)
        b.instructions[:] = keep
    return removed


def prune_mm_updates(nc):
    """Remove PE-matmul sem updates that no waiter's threshold references.

    Every tile-emitted matmul carries a `PE_sem++@complete`; the hardware
    retires that sem write on the PE sequencer, and when the next
    instruction is another matmul the write stalls its issue (~40ns
    measured).  Waiters reference only ~100 distinct cumulative counts, so
    all other increments are unobservable.  Keep an update only at
    referenced positions and renumber every wait threshold to the new
    cumulative count.  Kept updates remain on the same instructions, so
    every waiter still unblocks at exactly the same matmul completion as
    before.
    """
    from concourse import mybir

    upd_by = {}
    for f in nc.m.functions:
        for b in f.blocks:
            for i in b.instructions:
                si = i.sync_info
                if not si:
                    continue
                for u in si.on_update:
                    rec = upd_by.setdefault(u.id, {"mm": 0, "other": 0})
                    if (
                        i.engine == mybir.EngineType.PE
                        and isinstance(i, mybir.InstMatmult)
                        and u.update_mode == "sem-inc"
                        and (u.update_value or 1) == 1
                    ):
                        rec["mm"] += 1
                    else:
                        rec["other"] += 1
    candidates = [s for s, r in upd_by.items() if r["mm"] > 0 and r["other"] == 0]

    total_removed = 0
    for sem_id in candidates:
        updaters = []
        waits = []
        ok = True
        for f in nc.m.functions:
            for b in f.blocks:
                for i in b.instructions:
                    si = i.sync_info
                    if not si:
                        continue
                    for u in si.on_update:
                        if u.id == sem_id:
                            updaters.append(i)
                    for w in si.on_wait:
                        if w.id == sem_id:
                            if (
                                w.wait_mode != "sem-ge-imm"
                                or w.wait_reg is not None
                                or w.wait_value is None
                            ):
                                ok = False
                            waits.append(w)
        if not ok or not waits:
            continue
        n = len(updaters)
        keep = set()
        for w in waits:
            t = w.wait_value
            if t < 1 or t > n:
                ok = False
                break
            keep.add(t)
        if not ok:
            continue
        keep.add(n)  # preserve the final count for any implicit consumers
        sorted_keep = sorted(keep)
        new_count = {pos: rank for rank, pos in enumerate(sorted_keep, start=1)}
        for w in waits:
            w.wait_value = new_count[w.wait_value]
        for pos, inst in enumerate(updaters, start=1):
            if pos not in keep:
                si = inst.sync_info
                si.on_update = [u for u in si.on_update if u.id != sem_id]
                total_removed += 1
    return total_removed


def get_nc():
    global _NC
    if _NC is None:
        _NC = build_nc()
    return _NC


def _prep_shared(W, b, A, Bm, Wr):
    # W (OUT, IN) -> per feature-half h: wd[ot, p, k, o] = Wh[ot*128+o, k*128+p]
    wds = []
    bts = []
    bt_full = np.concatenate(
        [Bm.transpose(0, 2, 1).reshape(ER, OUT), b[None, :]], axis=0
    ).astype(np.float16)  # (33, OUT)
    for h in range(FH):
        Wh = W[h * OPC : (h + 1) * OPC]
        wds.append(
            np.ascontiguousarray(
                Wh.reshape(OT, P, KT, P).transpose(0, 3, 2, 1).astype(np.float16)
            )
        )
        bts.append(np.ascontiguousarray(bt_full[:, h * OPC : (h + 1) * OPC]))
    # [A (E,R,IN) flattened; Wr (E,IN)] -> art[p, k, j] = AR[j, k*128+p]
    ar = np.concatenate([A.reshape(ER, IN), Wr], axis=0)  # (36, IN)
    artd = np.ascontiguousarray(
        ar.T.reshape(KT, P, ER + E).transpose(1, 0, 2).astype(np.float16)
    )
    sel = np.zeros((E, ER), np.float32)
    for e in range(E):
        sel[e, e * R : (e + 1) * R] = SCALE
    return wds, artd, bts, sel


def _prep_x_shard(xt, g):
    xs = xt[g * TPC : (g + 1) * TPC]  # (TPC, IN)
    return np.ascontiguousarray(
        xs.T.reshape(KT, P, TPC).transpose(1, 0, 2).astype(np.float16)
    )


def make_in_maps(x, W, b, A, Bm, Wr):
    xt = np.asarray(x, np.float32).reshape(TOK, IN)
    wds, artd, bts, sel = _prep_shared(
        np.asarray(W, np.float32),
        np.asarray(b, np.float32),
        np.asarray(A, np.float32),
        np.asarray(Bm, np.float32),
        np.asarray(Wr, np.float32),
    )
    xds = [_prep_x_shard(xt, g) for g in range(TG)]
    return [
        {
            "xd": xds[c // FH],
            "wd": wds[c % FH],
            "artd": artd,
            "btd": bts[c % FH],
            "seld": sel,
        }
        for c in range(N_CORES)
    ]


def gather_out(results):
    # core c = (token group c//FH, feature half c%FH); od is (OPC, TPC)
    out = np.empty((TOK, OUT), np.float32)
    for c, r in enumerate(results):
        g, h = c // FH, c % FH
        out[g * TPC : (g + 1) * TPC, h * OPC : (h + 1) * OPC] = r["od"].T
    return out.reshape(B, S, OUT)


def kernel(x, W, b, A, Bm, Wr, _trace=False):
    nc = get_nc()
    in_maps = make_in_maps(x, W, b, A, Bm, Wr)
    res = bass_utils.run_bass_kernel_spmd(
        nc, in_maps, core_ids=list(range(N_CORES)), trace=_trace
    )
    out = gather_out(res.results)
    if _trace:
        return out, res
    return out


# revision 16
# speedup vs baseline: 1.3220x; 1.3220x over previous
"""MoE QLoRA linear kernel for Trainium2 (8 NeuronCores, data-parallel over tokens).

Computes, for x:(B,S,IN) f32:
    base  = x @ W.T + b
    gates = softmax(x @ Wr.T)                       # (tok, E)
    proj  = x @ A[e].T                              # (tok, E, R)
    out   = base + sum_e SCALE * gates[...,e] * (proj[...,e,:] @ Bm[e].T)

Key algebraic fold: the gated expert mix is a single rank-(E*R) matmul:
    wproj[t, er] = SCALE * gates[t, e] * proj[t, er]          (er = e*R+r)
    lora[t, o]   = sum_er wproj[t, er] * Bcat[er, o]          (Bcat[er,o] = Bm[e,o,r])
and the bias b is folded in as an extra contraction row (wproj row of ones,
Bcat row = b), so base+lora+bias all accumulate in one PSUM group on the PE.

Per-core kernel (1024 tokens), everything oriented (feature-partition, token-free):
  phase 1: PSUM(36,512) = [A;Wr]^T-stationary matmuls over 32 k-tiles ->
           proj rows 0..31, router logits rows 32..35; softmax via exp +
           PE ones-matmul partition reductions/broadcasts; wproj written fp16.
           The o-tile-0 base k-loop is emitted between the proj matmuls and
           the gating chain so the PE stays busy while ACT/DVE run softmax.
  phase 2: for each of 32 o-tiles: out(128o, t) = W-tile-stationary matmul
           over 32 k-tiles + one lora matmul (k=33) accumulated into PSUM,
           copy to SBUF, DMA out as (OUT, tok); host transposes back.

All matmul inputs are fp16 (host-cast; PE runs fp16 at full bf16 rate,
fp32 PSUM accumulation). Host pre-tiles all layouts so every DMA is
contiguous and the kernel needs zero on-chip transposes.

Perf findings (all hardware-measured via rep-scaling slopes on the base
phase, 2112 N=512 matmuls/rep):
  * Per-matmul cost is ~255-266ns in EVERY instruction-stream variant
    tried: LDWEIGHTS-per-MM (baseline), deduped LDWEIGHTS (LDW,MM,MM),
    per-MM sem updates pruned, and 4-long same-stationary chains under a
    2048-token resharding.  512 cycles at ~2.0GHz is 256ns: under
    sustained 8-core load the chip sits in the P0 power state (PE ~2.0GHz,
    not the 2.4GHz the cost model assumes), and that — not LDWEIGHTS, sem
    updates, or weight-reload drains — is the ~560us-vs-450us gap to the
    cost model.  Slopes also grow ~10-15% across back-to-back trials
    (progressive throttling), so test.py cools down between trials and
    reports the min.
  * Deduping LDWEIGHTS *without* pruning sem updates is ~40us WORSE: a
    sem write retires for free under a following LDWEIGHTS but stalls a
    bare MM->MM turnaround (~91ns vs ~53ns measured).  With updates
    pruned the two variants are equal; both post-compile passes are kept
    (measurably never worse, best observed trials ~529us vs ~545us).
  * A 2048tok x 2048of resharding (4-long same-stationary chains) was
    built and correct on hardware but did not beat this structure: the
    chain amortization predicted at 2.4GHz does not materialize at the
    P0-throttled rate, and duplicating phase 1 per feature-half costs
    ~26us of model-side full-kernel time.
  * Matmul count (2182) is at the hardware floor (m<=128, n<=512/PSUM
    bank, k<=128); fp8 (e4m3 DoubleRow for 2x rate, or e3m4 weights) fails
    the 2e-2 rel-err budget (measured 2.4-2.8% in numpy simulation).
  * PE-engine idle is only startup DMA (~11us, both alternate DMA rings
    measured worse) and the framework tail drain (~5us).
  * Walrus runs with --enable-ldw-opt=false (=true crashes codegen), so
    every emitted matmul arrives as a standalone InstLdweights + a
    non-self-loading InstMatmult; the dedup pass operates on that form.
"""

import numpy as np

import concourse.bass as bass
import concourse.tile as tile
from concourse import bacc, mybir
from concourse import bass_utils

# Problem shape (hardcoded; kernel.py must be self-contained)
B, S, IN, OUT, E, R = 4, 2048, 4096, 4096, 4, 8
SCALE = 16.0 / 8.0
N_CORES = 8
TOK = B * S                  # 8192 tokens
TPC = TOK // N_CORES         # 1024 tokens per core
P = 128                      # partitions
KT = IN // P                 # 32 k-tiles (contraction)
OT = OUT // P                # 32 output tiles
NSLAB = 512                  # moving-operand free size (PSUM bank = 512 f32)
NS = TPC // NSLAB            # 2 token slabs per core
ER = E * R                   # 32 low-rank rows
ERA = ER + 1                 # +1 ones row (bias fold)

F16 = mybir.dt.float16
F32 = mybir.dt.float32

_NC = None

# Optional post-compile transform hook (used by experiments; None in prod).
POST_COMPILE = None


def build_nc(reps=1, ns=NS):
    NS_ = ns
    nc = bacc.Bacc("TRN2", target_bir_lowering=False, debug=False)

    xd = nc.dram_tensor("xd", [P, KT, TPC], F16, kind="ExternalInput")
    wd = nc.dram_tensor("wd", [OT, P, KT, P], F16, kind="ExternalInput")
    artd = nc.dram_tensor("artd", [P, KT, ER + E], F16, kind="ExternalInput")
    btd = nc.dram_tensor("btd", [ERA, OUT], F16, kind="ExternalInput")
    seld = nc.dram_tensor("seld", [E, ER], F32, kind="ExternalInput")
    od = nc.dram_tensor("od", [OUT, TPC], F32, kind="ExternalOutput")

    with tile.TileContext(nc) as tc:
        with (
            tc.tile_pool(name="consts", bufs=1) as consts,
            tc.tile_pool(name="wpool", bufs=3) as wpool,
            tc.tile_pool(name="opool", bufs=3) as opool,
            tc.tile_pool(name="small", bufs=2) as small,
            tc.tile_pool(name="psum_proj", bufs=1, space="PSUM") as psum_proj,
            tc.tile_pool(name="psum_base", bufs=2, space="PSUM") as psum_base,
        ):
            art_sb = consts.tile([P, KT, ER + E], F16)
            nc.sync.dma_start(out=art_sb[:], in_=artd[:])
            bt_sb = consts.tile([ERA, OUT], F16)
            nc.sync.dma_start(out=bt_sb[:], in_=btd[:])
            sel_sb = consts.tile([E, ER], F32)
            nc.sync.dma_start(out=sel_sb[:], in_=seld[:])

            w_tiles = {}

            def load_w(ot):
                w_sb = wpool.tile([P, KT, P], F16, tag="w", name="w_sb")
                nc.sync.dma_start(out=w_sb[:], in_=wd[ot])
                w_tiles[ot] = w_sb

            # first two W tiles before the bulk x load: o-tile 0 can start
            # as soon as phase-1 finishes on the PE
            load_w(0)
            load_w(1)

            # Resident activations: x^T tiled (p=i%128, k=i//128, t), fp16, 8 MiB.
            x_sb = consts.tile([P, KT, TPC], F16)
            for k in range(KT):
                nc.sync.dma_start(out=x_sb[:, k, :], in_=xd[:, k, :])

            ones_e1 = consts.tile([E, 1], F32)
            nc.vector.memset(ones_e1[:], 1.0)
            ones_1e = consts.tile([1, E], F32)
            nc.vector.memset(ones_1e[:], 1.0)
            # Gated low-rank projection, fp16, rows 0..31 = wproj, row 32 = ones.
            wp_sb = consts.tile([ERA, TPC], F16)
            nc.vector.memset(wp_sb[ER : ER + 1, :], 1.0)

            # ---------- phase 1: proj + router matmuls ----------
            pps = []
            for t in range(NS_):
                tsl = slice(t * NSLAB, (t + 1) * NSLAB)
                # rows 0..31: proj^T (er, t); rows 32..35: router logits (e, t)
                pp = psum_proj.tile(
                    [ER + E, NSLAB], F32, tag=f"pp{t}", name=f"pp{t}"
                )
                for k in range(KT):
                    nc.tensor.matmul(
                        pp[:],
                        art_sb[:, k, :],
                        x_sb[:, k, tsl],
                        start=(k == 0),
                        stop=(k == KT - 1),
                    )
                pps.append(pp)

            def gating(t):
                # softmax over the 4 expert rows (no max-sub: |logit| < ~8),
                # partition reductions/broadcasts done with tiny PE matmuls
                tsl = slice(t * NSLAB, (t + 1) * NSLAB)
                pp = pps[t]
                e_sb = small.tile([E, NSLAB], F32, tag="e", name="e_sb")
                nc.scalar.activation(
                    e_sb[:], pp[ER : ER + E, :], mybir.ActivationFunctionType.Exp
                )
                s_ps = psum_proj.tile([1, NSLAB], F32, tag="gat", name="s_ps")
                nc.tensor.matmul(s_ps[:], ones_e1[:], e_sb[:])  # sum_e exp
                r_sb = small.tile([1, NSLAB], F32, tag="r", name="r_sb")
                nc.vector.reciprocal(r_sb[:], s_ps[:])
                r4_ps = psum_proj.tile([E, NSLAB], F32, tag="gat", name="r4_ps")
                nc.tensor.matmul(r4_ps[:], ones_1e[:], r_sb[:])  # bcast to 4 rows
                g4_sb = small.tile([E, NSLAB], F32, tag="g4", name="g4_sb")
                nc.vector.tensor_mul(g4_sb[:], e_sb[:], r4_ps[:])
                # (SCALE * gate)[er, t] via 0/1*SCALE selection matmul
                g32_ps = psum_proj.tile([ER, NSLAB], F32, tag="gat", name="g32_ps")
                nc.tensor.matmul(g32_ps[:], sel_sb[:], g4_sb[:])
                # walrus: tensor_tensor may read at most one operand from PSUM
                g32_sb = small.tile([ER, NSLAB], F32, tag="g32s", name="g32_sb")
                nc.vector.tensor_copy(g32_sb[:], g32_ps[:])
                nc.vector.tensor_mul(wp_sb[0:ER, tsl], pp[0:ER, :], g32_sb[:])

            # ---------- phase 2: base matmul + lora + bias ----------
            def base_kloop(ot):
                if ot not in w_tiles:
                    load_w(ot)
                pots = [
                    psum_base.tile([P, NSLAB], F32, tag=f"po{t}", name=f"po{t}")
                    for t in range(NS_)
                ]
                for k in range(KT):
                    for t in range(NS_):
                        nc.tensor.matmul(
                            pots[t][:],
                            w_tiles[ot][:, k, :],
                            x_sb[:, k, t * NSLAB : (t + 1) * NSLAB],
                            start=(k == 0),
                            stop=False,
                        )
                return pots

            def base_tail(ot, pots):
                osl = slice(ot * P, (ot + 1) * P)
                for t in range(NS_):
                    nc.tensor.matmul(
                        pots[t][:],
                        bt_sb[:, osl],
                        wp_sb[:, t * NSLAB : (t + 1) * NSLAB],
                        start=False,
                        stop=True,
                    )
                o_sb = opool.tile([P, TPC], F32, tag="o", name="o_sb")
                for t in range(NS_):
                    nc.vector.tensor_copy(
                        o_sb[:, t * NSLAB : (t + 1) * NSLAB], pots[t][:]
                    )
                nc.sync.dma_start(out=od[osl, :], in_=o_sb[:])
                del w_tiles[ot]

            for rep in range(reps):
                if rep == 0:
                    # o-tile 0's k-loop keeps the PE busy during the gating chain
                    pots0 = base_kloop(0)
                    for t in range(NS_):
                        gating(t)
                    base_tail(0, pots0)
                    start_ot = 1
                else:
                    start_ot = 0
                for ot in range(start_ot, OT):
                    pots = base_kloop(ot)
                    base_tail(ot, pots)

    nc.compile()
    if POST_COMPILE is not None:
        POST_COMPILE(nc)
    else:
        prune_mm_updates(nc)
        dedup_ldweights(nc)
    return nc


def dedup_ldweights(nc):
    """Delete redundant consecutive InstLdweights from the PE stream.

    After bacc lowering every matmul is a standalone InstLdweights followed
    by a non-self-loading InstMatmult.  When two MMs in a row use the same
    stationary tile (the two 512-token slabs of one (o-tile, k) pair), the
    second load is a no-op reload of weights already in the array.  Deleting
    it is safe when (a) its full AP signature matches the immediately
    preceding InstLdweights, (b) only non-self-loading InstMatmults sit
    between them (nothing else touched the array or SBUF ordering), and
    (c) the load carries no semaphore waits/updates.  Any rewrite of the
    underlying SBUF region is ordered after the *later* consumers by the
    tile framework's region tracking, so the weight contents cannot change
    between the two loads.
    """
    from concourse import mybir

    def sig(i):
        ap = i.ins[0]
        return (
            ap.memref,
            ap.offset,
            str(ap.ap),
            str(ap.dtype),
            str(i.tile_position),
            str(i.tile_size),
            str(i.perf_mode),
            str(i.is_transpose),
        )

    removed = 0
    for b in nc.m.functions[0].blocks:
        last = None
        keep = []
        for i in b.instructions:
            if i.engine != mybir.EngineType.PE:
                keep.append(i)
                continue
            if isinstance(i, mybir.InstLdweights):
                si = i.sync_info
                clean = si is None or (not si.on_wait and not si.on_update)
                if clean and last is not None and sig(i) == last:
                    removed += 1
                    continue  # drop the redundant reload
                last = sig(i)
                keep.append(i)
            elif (
                isinstance(i, mybir.InstMatmult)
                and getattr(i, "ldweights", None) is False
            ):
                keep.append(i)
            else:
                last = None
                keep.append(i)
        b.instructions[:] = keep
    return removed


def prune_mm_updates(nc):
    """Remove PE-matmul sem updates that no waiter's threshold references.

    Every tile-emitted matmul carries a `PE_sem++@complete`; the hardware
    retires that sem write on the PE sequencer, and when the next instruction
    is another matmul the write stalls its issue (~40ns measured: bare MM->MM
    gap 91ns vs 53ns with an LDWEIGHTS between).  Waiters (DVE copies, DMA
    recycles) reference only ~100 distinct cumulative counts, so all other
    increments are unobservable.  Keep an update only at referenced
    positions and renumber every wait threshold to the new cumulative count.
    Kept updates remain on the same instructions, so every waiter still
    unblocks at exactly the same matmul completion as before.
    """
    from concourse import mybir

    SEM_CANDIDATES = []
    # find sems updated exclusively by PE InstMatmult via sem-inc
    upd_by = {}
    for f in nc.m.functions:
        for b in f.blocks:
            for i in b.instructions:
                si = i.sync_info
                if not si:
                    continue
                for u in si.on_update:
                    rec = upd_by.setdefault(u.id, {"mm": 0, "other": 0})
                    if (
                        i.engine == mybir.EngineType.PE
                        and isinstance(i, mybir.InstMatmult)
                        and u.update_mode == "sem-inc"
                        and (u.update_value or 1) == 1
                    ):
                        rec["mm"] += 1
                    else:
                        rec["other"] += 1
    for sem_id, rec in upd_by.items():
        if rec["mm"] > 0 and rec["other"] == 0:
            SEM_CANDIDATES.append(sem_id)

    total_removed = 0
    for sem_id in SEM_CANDIDATES:
        # ordered updater positions (PE program order across blocks)
        updaters = []  # instruction refs in order
        waits = []  # (SyncWait refs)
        ok = True
        for f in nc.m.functions:
            for b in f.blocks:
                for i in b.instructions:
                    si = i.sync_info
                    if not si:
                        continue
                    for u in si.on_update:
                        if u.id == sem_id:
                            updaters.append(i)
                    for w in si.on_wait:
                        if w.id == sem_id:
                            if (
                                w.wait_mode != "sem-ge-imm"
                                or w.wait_reg is not None
                                or w.wait_value is None
                            ):
                                ok = False
                            waits.append(w)
        if not ok or not waits:
            continue
        n = len(updaters)
        keep = set()
        for w in waits:
            t = w.wait_value
            if t < 1 or t > n:
                ok = False
                break
            keep.add(t)
        if not ok:
            continue
        keep.add(n)  # preserve the final count for any implicit consumers
        # renumber
        sorted_keep = sorted(keep)
        new_count = {}
        for rank, pos in enumerate(sorted_keep, start=1):
            new_count[pos] = rank
        for w in waits:
            w.wait_value = new_count[w.wait_value]
        for pos, inst in enumerate(updaters, start=1):
            if pos not in keep:
                si = inst.sync_info
                si.on_update = [
                    u for u in si.on_update if u.id != sem_id
                ]
                total_removed += 1
    return total_removed


def consolidate_mm_updates(nc):
    """Move per-matmul semaphore completion updates to accumulation-group ends.

    Every matmul the tile framework emits carries a `sem++@complete` update;
    on hardware each update costs ~26ns of PE sequencer time (see the
    pack-tail model in the tensor-engine guide), which over ~2100 matmuls is
    ~55us of pure overhead.  Matmuls complete in program order, so moving the
    increments of the stop=False matmuls of an accumulation group onto the
    group's final stop=True matmul (with a summed update_value) is
    semantically conservative: every waiter still sees the same final counts,
    just potentially a few instructions later.  Waiters whose thresholds fall
    mid-group (e.g. the W-tile DMA recycling a pool buffer) unblock at the
    group end instead — with bufs=3 prefetch headroom that slack is never on
    the critical path.  Only runs of {InstMatmult, InstLdweights} are
    touched; a segment that does not end in a stop=True matmul is left as-is.
    """
    from concourse import mybir

    moved = 0
    for b in nc.m.functions[0].blocks:
        pending = []  # stripped SyncUpdate objects awaiting a stop=True MM
        stripped = []  # (instruction, saved updates) to restore if no flush
        for i in b.instructions:
            if i.engine != mybir.EngineType.PE:
                continue
            if isinstance(i, mybir.InstLdweights):
                continue
            if isinstance(i, mybir.InstMatmult):
                si = i.sync_info
                if i.stop_tensor_calc:
                    if pending:
                        ups = list(si.on_update) if si is not None else []
                        for u in pending:
                            for tgt in ups:
                                if (
                                    tgt.sync_type == u.sync_type
                                    and tgt.id == u.id
                                    and tgt.update_mode == u.update_mode
                                ):
                                    tgt.update_value = (
                                        tgt.update_value or 1
                                    ) + (u.update_value or 1)
                                    break
                            else:
                                ups.append(u)
                        if si is None:
                            i.sync_info = mybir.SyncInfo(
                                on_wait=[], on_update=ups
                            )
                        else:
                            si.on_update = ups
                        pending = []
                        stripped = []
                else:
                    if si is not None and si.on_update:
                        pending.extend(si.on_update)
                        moved += len(si.on_update)
                        stripped.append((i, list(si.on_update)))
                        si.on_update = []
            else:
                # A non-MM PE instruction (event sem, drain, branch) ends the
                # run; restore any updates not yet flushed so ordering
                # relative to this instruction is preserved.
                for inst, ups in stripped:
                    inst.sync_info.on_update = ups
                    moved -= len(ups)
                pending = []
                stripped = []
        for inst, ups in stripped:
            inst.sync_info.on_update = ups
            moved -= len(ups)
    return moved


def get_nc():
    global _NC
    if _NC is None:
        _NC = build_nc()
    return _NC


def _prep_shared(W, b, A, Bm, Wr):
    # W (OUT, IN) -> wd[ot, p, k, o] = W[ot*128+o, k*128+p], fp16, contiguous
    wd = np.ascontiguousarray(
        W.reshape(OT, P, KT, P).transpose(0, 3, 2, 1).astype(np.float16)
    )
    # [A (E,R,IN) flattened; Wr (E,IN)] -> art[p, k, j] = AR[j, k*128+p]
    ar = np.concatenate([A.reshape(ER, IN), Wr], axis=0)  # (36, IN)
    artd = np.ascontiguousarray(
        ar.T.reshape(KT, P, ER + E).transpose(1, 0, 2).astype(np.float16)
    )
    # Bcat rows er = Bm[e,:,r]; row 32 = bias
    bt = np.concatenate([Bm.transpose(0, 2, 1).reshape(ER, OUT), b[None, :]], axis=0)
    btd = np.ascontiguousarray(bt.astype(np.float16))
    sel = np.zeros((E, ER), np.float32)
    for e in range(E):
        sel[e, e * R : (e + 1) * R] = SCALE
    return wd, artd, btd, sel


def _prep_x_shard(xt, c):
    xs = xt[c * TPC : (c + 1) * TPC]  # (TPC, IN)
    return np.ascontiguousarray(
        xs.T.reshape(KT, P, TPC).transpose(1, 0, 2).astype(np.float16)
    )


def make_in_maps(x, W, b, A, Bm, Wr):
    xt = np.asarray(x, np.float32).reshape(TOK, IN)
    wd, artd, btd, sel = _prep_shared(
        np.asarray(W, np.float32),
        np.asarray(b, np.float32),
        np.asarray(A, np.float32),
        np.asarray(Bm, np.float32),
        np.asarray(Wr, np.float32),
    )
    return [
        {
            "xd": _prep_x_shard(xt, c),
            "wd": wd,
            "artd": artd,
            "btd": btd,
            "seld": sel,
        }
        for c in range(N_CORES)
    ]


def gather_out(results):
    # per-core od is (OUT, TPC); tokens are sharded contiguously
    return np.concatenate([r["od"].T for r in results], axis=0).reshape(B, S, OUT)


def kernel(x, W, b, A, Bm, Wr, _trace=False):
    nc = get_nc()
    in_maps = make_in_maps(x, W, b, A, Bm, Wr)
    res = bass_utils.run_bass_kernel_spmd(
        nc, in_maps, core_ids=list(range(N_CORES)), trace=_trace
    )
    out = gather_out(res.results)
    if _trace:
        return out, res
    return out



# revision 18
# speedup vs baseline: 1.3359x; 1.0105x over previous
"""MoE QLoRA linear kernel for Trainium2 (8 NeuronCores, data-parallel over tokens).

Computes, for x:(B,S,IN) f32:
    base  = x @ W.T + b
    gates = softmax(x @ Wr.T)                       # (tok, E)
    proj  = x @ A[e].T                              # (tok, E, R)
    out   = base + sum_e SCALE * gates[...,e] * (proj[...,e,:] @ Bm[e].T)

Key algebraic fold: the gated expert mix is a single rank-(E*R) matmul:
    wproj[t, er] = SCALE * gates[t, e] * proj[t, er]          (er = e*R+r)
    lora[t, o]   = sum_er wproj[t, er] * Bcat[er, o]          (Bcat[er,o] = Bm[e,o,r])
and the bias b is folded in as an extra contraction row (wproj row of ones,
Bcat row = b), so base+lora+bias all accumulate in one PSUM group on the PE.

Per-core kernel (1024 tokens), everything oriented (feature-partition, token-free):
  phase 1: PSUM(36,512) = [A;Wr]^T-stationary matmuls over 32 k-tiles ->
           proj rows 0..31, router logits rows 32..35; softmax via exp +
           PE ones-matmul partition reductions/broadcasts; wproj written fp16.
           The o-tile-0 base k-loop is emitted between the proj matmuls and
           the gating chain so the PE stays busy while ACT/DVE run softmax.
  phase 2: for each of 32 o-tiles: out(128o, t) = W-tile-stationary matmul
           over 32 k-tiles + one lora matmul (k=33) accumulated into PSUM,
           copy to SBUF, DMA out as (OUT, tok); host transposes back.

All matmul inputs are fp16 (host-cast; PE runs fp16 at full bf16 rate,
fp32 PSUM accumulation). Host pre-tiles all layouts so every DMA is
contiguous and the kernel needs zero on-chip transposes.

Perf findings (all hardware-measured via rep-scaling slopes on the base
phase, 2112 N=512 matmuls/rep):
  * Per-matmul cost is ~255-266ns in EVERY instruction-stream variant
    tried: LDWEIGHTS-per-MM (baseline), deduped LDWEIGHTS (LDW,MM,MM),
    per-MM sem updates pruned, and 4-long same-stationary chains under a
    2048-token resharding.  512 cycles at ~2.0GHz is 256ns: under
    sustained 8-core load the chip sits in the P0 power state (PE ~2.0GHz,
    not the 2.4GHz the cost model assumes), and that — not LDWEIGHTS, sem
    updates, or weight-reload drains — is the ~560us-vs-450us gap to the
    cost model.  Slopes also grow ~10-15% across back-to-back trials
    (progressive throttling), so test.py cools down between trials and
    reports the min.
  * Deduping LDWEIGHTS *without* pruning sem updates is ~40us WORSE: a
    sem write retires for free under a following LDWEIGHTS but stalls a
    bare MM->MM turnaround (~91ns vs ~53ns measured).  With updates
    pruned the two variants are equal; both post-compile passes are kept
    (measurably never worse, best observed trials ~529us vs ~545us).
  * A 2048tok x 2048of resharding (4-long same-stationary chains) was
    built and correct on hardware but did not beat this structure: the
    chain amortization predicted at 2.4GHz does not materialize at the
    P0-throttled rate, and duplicating phase 1 per feature-half costs
    ~26us of model-side full-kernel time.
  * Matmul count (2182) is at the hardware floor (m<=128, n<=512/PSUM
    bank, k<=128); fp8 (e4m3 DoubleRow for 2x rate, or e3m4 weights) fails
    the 2e-2 rel-err budget (measured 2.4-2.8% in numpy simulation).
  * PE-engine idle is only startup DMA (~11us, both alternate DMA rings
    measured worse) and the framework tail drain (~5us).
  * Walrus runs with --enable-ldw-opt=false (=true crashes codegen), so
    every emitted matmul arrives as a standalone InstLdweights + a
    non-self-loading InstMatmult; the dedup pass operates on that form.
"""

import numpy as np

import concourse.bass as bass
import concourse.tile as tile
from concourse import bacc, mybir
from concourse import bass_utils

# Problem shape (hardcoded; kernel.py must be self-contained)
B, S, IN, OUT, E, R = 4, 2048, 4096, 4096, 4, 8
SCALE = 16.0 / 8.0
N_CORES = 8
TOK = B * S                  # 8192 tokens
TPC = TOK // N_CORES         # 1024 tokens per core
P = 128                      # partitions
KT = IN // P                 # 32 k-tiles (contraction)
OT = OUT // P                # 32 output tiles
NSLAB = 512                  # moving-operand free size (PSUM bank = 512 f32)
NS = TPC // NSLAB            # 2 token slabs per core
ER = E * R                   # 32 low-rank rows
ERA = ER + 1                 # +1 ones row (bias fold)

F16 = mybir.dt.float16
F32 = mybir.dt.float32

_NC = None

# Optional post-compile transform hook (used by experiments; None in prod).
POST_COMPILE = None


def build_nc(reps=1, ns=NS):
    NS_ = ns
    nc = bacc.Bacc("TRN2", target_bir_lowering=False, debug=False)

    xd = nc.dram_tensor("xd", [P, KT, TPC], F16, kind="ExternalInput")
    wd = nc.dram_tensor("wd", [OT, P, KT, P], F16, kind="ExternalInput")
    artd = nc.dram_tensor("artd", [P, KT, ER + E], F16, kind="ExternalInput")
    btd = nc.dram_tensor("btd", [ERA, OUT], F16, kind="ExternalInput")
    seld = nc.dram_tensor("seld", [E, ER], F32, kind="ExternalInput")
    od = nc.dram_tensor("od", [OUT, TPC], F32, kind="ExternalOutput")

    with tile.TileContext(nc) as tc:
        with (
            tc.tile_pool(name="consts", bufs=1) as consts,
            tc.tile_pool(name="wpool", bufs=3) as wpool,
            tc.tile_pool(name="opool", bufs=3) as opool,
            tc.tile_pool(name="small", bufs=2) as small,
            tc.tile_pool(name="psum_proj", bufs=1, space="PSUM") as psum_proj,
            tc.tile_pool(name="psum_base", bufs=2, space="PSUM") as psum_base,
        ):
            art_sb = consts.tile([P, KT, ER + E], F16)
            nc.sync.dma_start(out=art_sb[:], in_=artd[:])
            bt_sb = consts.tile([ERA, OUT], F16)
            nc.sync.dma_start(out=bt_sb[:], in_=btd[:])
            sel_sb = consts.tile([E, ER], F32)
            nc.sync.dma_start(out=sel_sb[:], in_=seld[:])

            w_tiles = {}

            def load_w(ot):
                w_sb = wpool.tile([P, KT, P], F16, tag="w", name="w_sb")
                nc.sync.dma_start(out=w_sb[:], in_=wd[ot])
                w_tiles[ot] = w_sb

            # first two W tiles before the bulk x load: o-tile 0 can start
            # as soon as phase-1 finishes on the PE
            load_w(0)
            load_w(1)

            # Resident activations: x^T tiled (p=i%128, k=i//128, t), fp16, 8 MiB.
            x_sb = consts.tile([P, KT, TPC], F16)
            for k in range(KT):
                nc.sync.dma_start(out=x_sb[:, k, :], in_=xd[:, k, :])

            ones_e1 = consts.tile([E, 1], F32)
            nc.vector.memset(ones_e1[:], 1.0)
            ones_1e = consts.tile([1, E], F32)
            nc.vector.memset(ones_1e[:], 1.0)
            # Gated low-rank projection, fp16, rows 0..31 = wproj, row 32 = ones.
            wp_sb = consts.tile([ERA, TPC], F16)
            nc.vector.memset(wp_sb[ER : ER + 1, :], 1.0)

            # ---------- phase 1: proj + router matmuls ----------
            pps = []
            for t in range(NS_):
                tsl = slice(t * NSLAB, (t + 1) * NSLAB)
                # rows 0..31: proj^T (er, t); rows 32..35: router logits (e, t)
                pp = psum_proj.tile(
                    [ER + E, NSLAB], F32, tag=f"pp{t}", name=f"pp{t}"
                )
                for k in range(KT):
                    nc.tensor.matmul(
                        pp[:],
                        art_sb[:, k, :],
                        x_sb[:, k, tsl],
                        start=(k == 0),
                        stop=(k == KT - 1),
                    )
                pps.append(pp)

            # Gating (softmax over the 4 expert rows; no max-sub: |logit| <
            # ~8) split into stages so its PE matmuls can be interleaved
            # between o-tile-0 k-iterations: each stage's DVE/ACT producer
            # then has ~8 k-iterations (~2us) of PE work to hide behind,
            # instead of stalling the in-order PE ~0.5us per stage.
            gst = {t: {} for t in range(NS_)}

            def g_exp(t):
                e_sb = small.tile([E, NSLAB], F32, tag="e", name="e_sb")
                nc.scalar.activation(
                    e_sb[:],
                    pps[t][ER : ER + E, :],
                    mybir.ActivationFunctionType.Exp,
                )
                gst[t]["e"] = e_sb

            def g_sum(t):
                s_ps = psum_proj.tile([1, NSLAB], F32, tag="gat", name="s_ps")
                nc.tensor.matmul(s_ps[:], ones_e1[:], gst[t]["e"][:])
                r_sb = small.tile([1, NSLAB], F32, tag="r", name="r_sb")
                nc.vector.reciprocal(r_sb[:], s_ps[:])
                gst[t]["r"] = r_sb

            def g_bcast(t):
                r4_ps = psum_proj.tile([E, NSLAB], F32, tag="gat", name="r4_ps")
                nc.tensor.matmul(r4_ps[:], ones_1e[:], gst[t]["r"][:])
                g4_sb = small.tile([E, NSLAB], F32, tag="g4", name="g4_sb")
                nc.vector.tensor_mul(g4_sb[:], gst[t]["e"][:], r4_ps[:])
                gst[t]["g4"] = g4_sb

            def g_sel(t):
                # (SCALE * gate)[er, t] via 0/1*SCALE selection matmul
                tsl = slice(t * NSLAB, (t + 1) * NSLAB)
                g32_ps = psum_proj.tile([ER, NSLAB], F32, tag="gat", name="g32_ps")
                nc.tensor.matmul(g32_ps[:], sel_sb[:], gst[t]["g4"][:])
                # walrus: tensor_tensor may read at most one operand from PSUM
                g32_sb = small.tile([ER, NSLAB], F32, tag="g32s", name="g32_sb")
                nc.vector.tensor_copy(g32_sb[:], g32_ps[:])
                nc.vector.tensor_mul(wp_sb[0:ER, tsl], pps[t][0:ER, :], g32_sb[:])

            def gating(t):
                g_exp(t)
                g_sum(t)
                g_bcast(t)
                g_sel(t)

            # ---------- phase 2: base matmul + lora + bias ----------
            def base_kloop(ot):
                if ot not in w_tiles:
                    load_w(ot)
                pots = [
                    psum_base.tile([P, NSLAB], F32, tag=f"po{t}", name=f"po{t}")
                    for t in range(NS_)
                ]
                for k in range(KT):
                    for t in range(NS_):
                        nc.tensor.matmul(
                            pots[t][:],
                            w_tiles[ot][:, k, :],
                            x_sb[:, k, t * NSLAB : (t + 1) * NSLAB],
                            start=(k == 0),
                            stop=False,
                        )
                return pots

            def base_tail(ot, pots):
                osl = slice(ot * P, (ot + 1) * P)
                for t in range(NS_):
                    nc.tensor.matmul(
                        pots[t][:],
                        bt_sb[:, osl],
                        wp_sb[:, t * NSLAB : (t + 1) * NSLAB],
                        start=False,
                        stop=True,
                    )
                o_sb = opool.tile([P, TPC], F32, tag="o", name="o_sb")
                for t in range(NS_):
                    nc.vector.tensor_copy(
                        o_sb[:, t * NSLAB : (t + 1) * NSLAB], pots[t][:]
                    )
                nc.sync.dma_start(out=od[osl, :], in_=o_sb[:])
                del w_tiles[ot]

            for rep in range(reps):
                if rep == 0:
                    # o-tile 0's k-loop keeps the PE busy during the gating
                    # chain, with the gating stages interleaved between
                    # k-iterations (before a k's slab pair, so the LDWEIGHTS
                    # dedup chase within each pair is unaffected).
                    g_exp(0)
                    if NS_ > 1:
                        g_exp(1)
                    sched = (
                        {4: (g_sum, 0), 8: (g_bcast, 0), 12: (g_sel, 0),
                         16: (g_sum, 1), 20: (g_bcast, 1), 24: (g_sel, 1)}
                        if NS_ > 1
                        else {4: (g_sum, 0), 12: (g_bcast, 0), 20: (g_sel, 0)}
                    )
                    if 0 not in w_tiles:
                        load_w(0)
                    pots0 = [
                        psum_base.tile(
                            [P, NSLAB], F32, tag=f"po{t}", name=f"po{t}"
                        )
                        for t in range(NS_)
                    ]
                    for k in range(KT):
                        if k in sched:
                            fn, t = sched[k]
                            fn(t)
                        for t in range(NS_):
                            nc.tensor.matmul(
                                pots0[t][:],
                                w_tiles[0][:, k, :],
                                x_sb[:, k, t * NSLAB : (t + 1) * NSLAB],
                                start=(k == 0),
                                stop=False,
                            )
                    base_tail(0, pots0)
                    start_ot = 1
                else:
                    start_ot = 0
                for ot in range(start_ot, OT):
                    pots = base_kloop(ot)
                    base_tail(ot, pots)

    nc.compile()
    if POST_COMPILE is not None:
        POST_COMPILE(nc)
    else:
        prune_mm_updates(nc)
        dedup_ldweights(nc)
    return nc


def dedup_ldweights(nc):
    """Delete redundant consecutive InstLdweights from the PE stream.

    After bacc lowering every matmul is a standalone InstLdweights followed
    by a non-self-loading InstMatmult.  When two MMs in a row use the same
    stationary tile (the two 512-token slabs of one (o-tile, k) pair), the
    second load is a no-op reload of weights already in the array.  Deleting
    it is safe when (a) its full AP signature matches the immediately
    preceding InstLdweights, (b) only non-self-loading InstMatmults sit
    between them (nothing else touched the array or SBUF ordering), and
    (c) the load carries no semaphore waits/updates.  Any rewrite of the
    underlying SBUF region is ordered after the *later* consumers by the
    tile framework's region tracking, so the weight contents cannot change
    between the two loads.
    """
    from concourse import mybir

    def sig(i):
        ap = i.ins[0]
        return (
            ap.memref,
            ap.offset,
            str(ap.ap),
            str(ap.dtype),
            str(i.tile_position),
            str(i.tile_size),
            str(i.perf_mode),
            str(i.is_transpose),
        )

    removed = 0
    for b in nc.m.functions[0].blocks:
        last = None
        keep = []
        for i in b.instructions:
            if i.engine != mybir.EngineType.PE:
                keep.append(i)
                continue
            if isinstance(i, mybir.InstLdweights):
                si = i.sync_info
                clean = si is None or (not si.on_wait and not si.on_update)
                if clean and last is not None and sig(i) == last:
                    removed += 1
                    continue  # drop the redundant reload
                last = sig(i)
                keep.append(i)
            elif (
                isinstance(i, mybir.InstMatmult)
                and getattr(i, "ldweights", None) is False
            ):
                keep.append(i)
            else:
                last = None
                keep.append(i)
        b.instructions[:] = keep
    return removed


def prune_mm_updates(nc):
    """Remove PE-matmul sem updates that no waiter's threshold references.

    Every tile-emitted matmul carries a `PE_sem++@complete`; the hardware
    retires that sem write on the PE sequencer, and when the next instruction
    is another matmul the write stalls its issue (~40ns measured: bare MM->MM
    gap 91ns vs 53ns with an LDWEIGHTS between).  Waiters (DVE copies, DMA
    recycles) reference only ~100 distinct cumulative counts, so all other
    increments are unobservable.  Keep an update only at referenced
    positions and renumber every wait threshold to the new cumulative count.
    Kept updates remain on the same instructions, so every waiter still
    unblocks at exactly the same matmul completion as before.
    """
    from concourse import mybir

    SEM_CANDIDATES = []
    # find sems updated exclusively by PE InstMatmult via sem-inc
    upd_by = {}
    for f in nc.m.functions:
        for b in f.blocks:
            for i in b.instructions:
                si = i.sync_info
                if not si:
                    continue
                for u in si.on_update:
                    rec = upd_by.setdefault(u.id, {"mm": 0, "other": 0})
                    if (
                        i.engine == mybir.EngineType.PE
                        and isinstance(i, mybir.InstMatmult)
                        and u.update_mode == "sem-inc"
                        and (u.update_value or 1) == 1
                    ):
                        rec["mm"] += 1
                    else:
                        rec["other"] += 1
    for sem_id, rec in upd_by.items():
        if rec["mm"] > 0 and rec["other"] == 0:
            SEM_CANDIDATES.append(sem_id)

    total_removed = 0
    for sem_id in SEM_CANDIDATES:
        # ordered updater positions (PE program order across blocks)
        updaters = []  # instruction refs in order
        waits = []  # (SyncWait refs)
        ok = True
        for f in nc.m.functions:
            for b in f.blocks:
                for i in b.instructions:
                    si = i.sync_info
                    if not si:
                        continue
                    for u in si.on_update:
                        if u.id == sem_id:
                            updaters.append(i)
                    for w in si.on_wait:
                        if w.id == sem_id:
                            if (
                                w.wait_mode != "sem-ge-imm"
                                or w.wait_reg is not None
                                or w.wait_value is None
                            ):
                                ok = False
                            waits.append(w)
        if not ok or not waits:
            continue
        n = len(updaters)
        keep = set()
        for w in waits:
            t = w.wait_value
            if t < 1 or t > n:
                ok = False
                break
            keep.add(t)
        if not ok:
            continue
        keep.add(n)  # preserve the final count for any implicit consumers
        # renumber
        sorted_keep = sorted(keep)
        new_count = {}
        for rank, pos in enumerate(sorted_keep, start=1):
            new_count[pos] = rank
        for w in waits:
            w.wait_value = new_count[w.wait_value]
        for pos, inst in enumerate(updaters, start=1):
            if pos not in keep:
                si = inst.sync_info
                si.on_update = [
                    u for u in si.on_update if u.id != sem_id
                ]
                total_removed += 1
    return total_removed


def consolidate_mm_updates(nc):
    """Move per-matmul semaphore completion updates to accumulation-group ends.

    Every matmul the tile framework emits carries a `sem++@complete` update;
    on hardware each update costs ~26ns of PE sequencer time (see the
    pack-tail model in the tensor-engine guide), which over ~2100 matmuls is
    ~55us of pure overhead.  Matmuls complete in program order, so moving the
    increments of the stop=False matmuls of an accumulation group onto the
    group's final stop=True matmul (with a summed update_value) is
    semantically conservative: every waiter still sees the same final counts,
    just potentially a few instructions later.  Waiters whose thresholds fall
    mid-group (e.g. the W-tile DMA recycling a pool buffer) unblock at the
    group end instead — with bufs=3 prefetch headroom that slack is never on
    the critical path.  Only runs of {InstMatmult, InstLdweights} are
    touched; a segment that does not end in a stop=True matmul is left as-is.
    """
    from concourse import mybir

    moved = 0
    for b in nc.m.functions[0].blocks:
        pending = []  # stripped SyncUpdate objects awaiting a stop=True MM
        stripped = []  # (instruction, saved updates) to restore if no flush
        for i in b.instructions:
            if i.engine != mybir.EngineType.PE:
                continue
            if isinstance(i, mybir.InstLdweights):
                continue
            if isinstance(i, mybir.InstMatmult):
                si = i.sync_info
                if i.stop_tensor_calc:
                    if pending:
                        ups = list(si.on_update) if si is not None else []
                        for u in pending:
                            for tgt in ups:
                                if (
                                    tgt.sync_type == u.sync_type
                                    and tgt.id == u.id
                                    and tgt.update_mode == u.update_mode
                                ):
                                    tgt.update_value = (
                                        tgt.update_value or 1
                                    ) + (u.update_value or 1)
                                    break
                            else:
                                ups.append(u)
                        if si is None:
                            i.sync_info = mybir.SyncInfo(
                                on_wait=[], on_update=ups
                            )
                        else:
                            si.on_update = ups
                        pending = []
                        stripped = []
                else:
                    if si is not None and si.on_update:
                        pending.extend(si.on_update)
                        moved += len(si.on_update)
                        stripped.append((i, list(si.on_update)))
                        si.on_update = []
            else:
                # A non-MM PE instruction (event sem, drain, branch) ends the
                # run; restore any updates not yet flushed so ordering
                # relative to this instruction is preserved.
                for inst, ups in stripped:
                    inst.sync_info.on_update = ups
                    moved -= len(ups)
                pending = []
                stripped = []
        for inst, ups in stripped:
            inst.sync_info.on_update = ups
            moved -= len(ups)
    return moved


def get_nc():
    global _NC
    if _NC is None:
        _NC = build_nc()
    return _NC


def _prep_shared(W, b, A, Bm, Wr):
    # W (OUT, IN) -> wd[ot, p, k, o] = W[ot*128+o, k*128+p], fp16, contiguous
    wd = np.ascontiguousarray(
        W.reshape(OT, P, KT, P).transpose(0, 3, 2, 1).astype(np.float16)
    )
    # [A (E,R,IN) flattened; Wr (E,IN)] -> art[p, k, j] = AR[j, k*128+p]
    ar = np.concatenate([A.reshape(ER, IN), Wr], axis=0)  # (36, IN)
    artd = np.ascontiguousarray(
        ar.T.reshape(KT, P, ER + E).transpose(1, 0, 2).astype(np.float16)
    )
    # Bcat rows er = Bm[e,:,r]; row 32 = bias
    bt = np.concatenate([Bm.transpose(0, 2, 1).reshape(ER, OUT), b[None, :]], axis=0)
    btd = np.ascontiguousarray(bt.astype(np.float16))
    sel = np.zeros((E, ER), np.float32)
    for e in range(E):
        sel[e, e * R : (e + 1) * R] = SCALE
    return wd, artd, btd, sel


def _prep_x_shard(xt, c):
    xs = xt[c * TPC : (c + 1) * TPC]  # (TPC, IN)
    return np.ascontiguousarray(
        xs.T.reshape(KT, P, TPC).transpose(1, 0, 2).astype(np.float16)
    )


def make_in_maps(x, W, b, A, Bm, Wr):
    xt = np.asarray(x, np.float32).reshape(TOK, IN)
    wd, artd, btd, sel = _prep_shared(
        np.asarray(W, np.float32),
        np.asarray(b, np.float32),
        np.asarray(A, np.float32),
        np.asarray(Bm, np.float32),
        np.asarray(Wr, np.float32),
    )
    return [
        {
            "xd": _prep_x_shard(xt, c),
            "wd": wd,
            "artd": artd,
            "btd": btd,
            "seld": sel,
        }
        for c in range(N_CORES)
    ]


def gather_out(results):
    # per-core od is (OUT, TPC); tokens are sharded contiguously
    return np.concatenate([r["od"].T for r in results], axis=0).reshape(B, S, OUT)


def kernel(x, W, b, A, Bm, Wr, _trace=False):
    nc = get_nc()
    in_maps = make_in_maps(x, W, b, A, Bm, Wr)
    res = bass_utils.run_bass_kernel_spmd(
        nc, in_maps, core_ids=list(range(N_CORES)), trace=_trace
    )
    out = gather_out(res.results)
    if _trace:
        return out, res
    return out

